# revision 6
# baseline (speedup 1.0000x reference)
"""Trainium2 Bass kernel for nn_MixedSparseTraditionalMLP.

Computes y2 = relu(x1 @ W_up) @ W_down and x2_save (top-10 least-zero
channels of relu(x1 @ W_up), per batch), where W_up/W_down are the dense
weights with their LoRA factors merged in on the host.

Strategy (8 NeuronCores, token-parallel):
  - 8192 tokens split 1024/core; cores (2b, 2b+1) share batch b.
  - fp32-grade matmuls via bf16 hi/lo split (3 bf16 matmuls per product,
    fp32 PSUM accumulation): y1 computed transposed [I, tokens] so the
    channel zero-count is a free-axis reduction and the second matmul and
    the top-k gather consume it directly as lhsT.
  - zero-counts pair-AllReduced, top-10 selected on device via packed
    values (counts*8192 + channel) with two max8 passes -> exact
    jax.lax.top_k tie-breaking (lower index wins).
  - gather rides as two extra matmuls per i-chunk against a one-hot
    selection matrix built on device.
  - y2 produced transposed [H, tokens]; host transposes back.
"""

import numpy as np
import ml_dtypes

HIDDEN = 2048
INTER = 8192
B = 4
S = 2048
NCORES = 8
TOK = (B * S) // NCORES  # tokens per core = 1024
TG = 512                 # token group (matmul free dim)
NG = TOK // TG           # 2
HCH = HIDDEN // 128      # 16 h-chunks
ICH = INTER // 128       # 64 i-chunks
K = 10

bf16 = ml_dtypes.bfloat16

_PROG_CACHE = {}


def _split(x):
    hi = x.astype(bf16)
    lo = (x - hi.astype(np.float32)).astype(bf16)
    return hi, lo


def _build_program():
    import concourse.bacc as bacc
    import concourse.mybir as mybir
    import concourse.tile as tile

    dt = mybir.dt
    nc = bacc.Bacc("TRN2", target_bir_lowering=False, debug=False)

    # ---- I/O ----
    # x1 transposed per-core slice, chunked: [HCH, 128(h), TOK]
    x1t_hi = nc.dram_tensor("x1t_hi", [HCH, 128, TOK], dt.bfloat16, kind="ExternalInput")
    x1t_lo = nc.dram_tensor("x1t_lo", [HCH, 128, TOK], dt.bfloat16, kind="ExternalInput")
    # W_up tiled: [ICH, 128(h within chunk), HCH*128(i)] -- see host packing
    wup_hi = nc.dram_tensor("wup_hi", [ICH, 128, HCH * 128], dt.bfloat16, kind="ExternalInput")
    wup_lo = nc.dram_tensor("wup_lo", [ICH, 128, HCH * 128], dt.bfloat16, kind="ExternalInput")
    # W_down tiled: [HCH, 128(i within chunk), ICH*128(h)]
    wdn_hi = nc.dram_tensor("wdn_hi", [HCH, 128, ICH * 128], dt.bfloat16, kind="ExternalInput")
    wdn_lo = nc.dram_tensor("wdn_lo", [HCH, 128, ICH * 128], dt.bfloat16, kind="ExternalInput")
    # iota over global channel index i = p + 128*c at [p, c]
    iota_i = nc.dram_tensor("iota_i", [128, ICH], dt.int32, kind="ExternalInput")
    iota_f = nc.dram_tensor("iota_f", [128, ICH], dt.float32, kind="ExternalInput")

    y2t = nc.dram_tensor("y2t", [HIDDEN, TOK], dt.float32, kind="ExternalOutput")
    savet = nc.dram_tensor("savet", [K, TOK], dt.float32, kind="ExternalOutput")
    cnt_out = nc.dram_tensor("cnt_out", [128, ICH], dt.float32, kind="ExternalOutput")
    idx_out = nc.dram_tensor("idx_out", [1, 16], dt.float32, kind="ExternalOutput")

    RELU = mybir.ActivationFunctionType.Relu
    COPY = mybir.ActivationFunctionType.Copy

    with tile.TileContext(nc) as tc:
        with (
            tc.tile_pool(name="const", bufs=1) as const,
            tc.tile_pool(name="dram", bufs=1, space="DRAM") as dpool,
        ):
            iotai_sb = const.tile([128, ICH], dt.int32)
            nc.sync.dma_start(iotai_sb[:], iota_i[:])
            iotaf_sb = const.tile([128, ICH], dt.float32)
            nc.sync.dma_start(iotaf_sb[:], iota_f[:])
            cnt_g = const.tile([128, ICH, NG], dt.float32)
            sel = const.tile([128, ICH, K], dt.bfloat16)

            # x2^T spill space: [NG, ICH, 128(i), TG]
            x2h_sp = dpool.tile([NG, ICH, 128, TG], dt.bfloat16)
            x2l_sp = dpool.tile([NG, ICH, 128, TG], dt.bfloat16)

            # ---------------- pass 1: y1^T = W_up^T-ish matmul, relu, counts, spill
            with (
                tc.tile_pool(name="p1", bufs=2) as p1,
                tc.tile_pool(name="p1ps", bufs=2, space="PSUM") as p1ps,
            ):
                for g in range(NG):
                    x1h_sb = p1.tile([128, HCH, TG], dt.bfloat16, tag="x1h")
                    x1l_sb = p1.tile([128, HCH, TG], dt.bfloat16, tag="x1l")
                    nc.sync.dma_start(
                        x1h_sb[:],
                        x1t_hi[:, :, g * TG:(g + 1) * TG].rearrange("hc p t -> p hc t"),
                    )
                    nc.sync.dma_start(
                        x1l_sb[:],
                        x1t_lo[:, :, g * TG:(g + 1) * TG].rearrange("hc p t -> p hc t"),
                    )
                    for ic in range(ICH):
                        wuh = p1.tile([128, HCH, 128], dt.bfloat16, tag="wuh")
                        wul = p1.tile([128, HCH, 128], dt.bfloat16, tag="wul")
                        nc.sync.dma_start(
                            wuh[:], wup_hi[ic].rearrange("p (hc i) -> p hc i", i=128)
                        )
                        nc.sync.dma_start(
                            wul[:], wup_lo[ic].rearrange("p (hc i) -> p hc i", i=128)
                        )
                        ps = p1ps.tile([128, TG], dt.float32, tag="p1psum")
                        for hc in range(HCH):
                            first = hc == 0
                            nc.tensor.matmul(
                                ps[:], lhsT=wuh[:, hc, :], rhs=x1h_sb[:, hc, :],
                                start=first, stop=False,
                            )
                            nc.tensor.matmul(
                                ps[:], lhsT=wul[:, hc, :], rhs=x1h_sb[:, hc, :],
                                start=False, stop=False,
                            )
                            nc.tensor.matmul(
                                ps[:], lhsT=wuh[:, hc, :], rhs=x1l_sb[:, hc, :],
                                start=False, stop=(hc == HCH - 1),
                            )
                        x2f = p1.tile([128, TG], dt.float32, tag="x2f")
                        nc.scalar.activation(x2f[:], ps[:], RELU)
                        x2h = p1.tile([128, TG], dt.bfloat16, tag="x2h")
                        nc.vector.tensor_copy(x2h[:], x2f[:])
                        x2l = p1.tile([128, TG], dt.bfloat16, tag="x2l")
                        nc.vector.tensor_sub(x2l[:], x2f[:], x2h[:])
                        mask = p1.tile([128, TG], dt.float32, tag="mask")
                        nc.vector.tensor_scalar(
                            mask[:], ps[:], 0.0, None, op0=mybir.AluOpType.is_le
                        )
                        nc.vector.reduce_sum(
                            out=cnt_g[:, ic, g:g + 1], in_=mask[:],
                            axis=mybir.AxisListType.X,
                        )
                        nc.sync.dma_start(x2h_sp[g, ic], x2h[:])
                        nc.sync.dma_start(x2l_sp[g, ic], x2l[:])

            # ---------------- counts allreduce + topk + one-hot selection
            with tc.tile_pool(name="tk", bufs=1) as tk:
                cnt_sum = tk.tile([128, ICH], dt.float32)
                nc.vector.tensor_add(cnt_sum[:], cnt_g[:, :, 0], cnt_g[:, :, 1])
                cnt_in = dpool.tile([128, ICH], dt.float32)
                cnt_ar = dpool.tile([128, ICH], dt.float32)
                nc.sync.dma_start(cnt_in[:], cnt_sum[:])
                nc.gpsimd.collective_compute(
                    "AllReduce",
                    mybir.AluOpType.add,
                    replica_groups=[[0, 1], [2, 3], [4, 5], [6, 7]],
                    ins=[cnt_in.opt()],
                    outs=[cnt_ar.opt()],
                )
                cnt_all = tk.tile([128, ICH], dt.float32)
                nc.sync.dma_start(cnt_all[:], cnt_ar[:])
                nc.sync.dma_start(cnt_out[:], cnt_all[:])
                pk = tk.tile([128, ICH], dt.float32)
                nc.vector.tensor_scalar_mul(pk[:], cnt_all[:], -8192.0)
                nc.vector.tensor_sub(pk[:], pk[:], iotaf_sb[:])
                pk_d = dpool.tile([1, 128 * ICH], dt.float32)
                nc.sync.dma_start(pk_d[0, :].rearrange("(p c) -> p c", c=ICH), pk[:])
                row = tk.tile([1, 128 * ICH], dt.float32)
                nc.sync.dma_start(row[:], pk_d[:])
                top16 = tk.tile([1, 16], dt.float32)
                nc.vector.max(top16[:, 0:8], row[:])
                nc.vector.match_replace(
                    out=row[:], in_to_replace=top16[:, 0:8], in_values=row[:],
                    imm_value=-1e30,
                )
                nc.vector.max(top16[:, 8:16], row[:])
                nc.sync.dma_start(idx_out[:], top16[:])
                pk10 = tk.tile([1, K], dt.float32)
                nc.vector.tensor_scalar_mul(pk10[:], top16[:, :K], -1.0)
                pk10_i = tk.tile([1, K], dt.int32)
                nc.vector.tensor_copy(pk10_i[:], pk10[:])
                idx_i = tk.tile([1, K], dt.int32)
                nc.vector.tensor_scalar(
                    idx_i[:], pk10_i[:], 8191, None, op0=mybir.AluOpType.bitwise_and
                )
                idxb = tk.tile([128, K], dt.int32)
                nc.gpsimd.partition_broadcast(idxb[:], idx_i[:])
                for ic in range(ICH):
                    nc.vector.tensor_tensor(
                        sel[:, ic, :],
                        iotai_sb[:, ic:ic + 1].to_broadcast([128, K]),
                        idxb[:],
                        mybir.AluOpType.is_equal,
                    )

            # ---------------- pass 2: y2^T + gather
            with (
                tc.tile_pool(name="p2", bufs=2) as p2,
                tc.tile_pool(name="p2x", bufs=1) as p2x,
                tc.tile_pool(name="p2ps", bufs=2, space="PSUM") as p2ps,
                tc.tile_pool(name="p2pss", bufs=1, space="PSUM") as p2pss,
            ):
                for g in range(NG):
                    x2h_sb = p2x.tile([128, ICH, TG], dt.bfloat16, tag="x2hsb")
                    x2l_sb = p2x.tile([128, ICH, TG], dt.bfloat16, tag="x2lsb")
                    for ic in range(ICH):
                        nc.sync.dma_start(x2h_sb[:, ic, :], x2h_sp[g, ic])
                        nc.sync.dma_start(x2l_sb[:, ic, :], x2l_sp[g, ic])
                    for hc in range(HCH):
                        psy = p2ps.tile([128, TG], dt.float32, tag="p2psum")
                        for half in range(2):
                            wdh = p2.tile([128, ICH // 2, 128], dt.bfloat16, tag="wdh")
                            wdl = p2.tile([128, ICH // 2, 128], dt.bfloat16, tag="wdl")
                            lo, hi_ = half * (ICH // 2) * 128, (half + 1) * (ICH // 2) * 128
                            nc.sync.dma_start(
                                wdh[:],
                                wdn_hi[hc][:, lo:hi_].rearrange("p (c h) -> p c h", h=128),
                            )
                            nc.sync.dma_start(
                                wdl[:],
                                wdn_lo[hc][:, lo:hi_].rearrange("p (c h) -> p c h", h=128),
                            )
                            for c2 in range(ICH // 2):
                                ic = half * (ICH // 2) + c2
                                first = ic == 0
                                nc.tensor.matmul(
                                    psy[:], lhsT=wdh[:, c2, :], rhs=x2h_sb[:, ic, :],
                                    start=first, stop=False,
                                )
                                nc.tensor.matmul(
                                    psy[:], lhsT=wdl[:, c2, :], rhs=x2h_sb[:, ic, :],
                                    start=False, stop=False,
                                )
                                nc.tensor.matmul(
                                    psy[:], lhsT=wdh[:, c2, :], rhs=x2l_sb[:, ic, :],
                                    start=False, stop=(ic == ICH - 1),
                                )
                        y2sb = p2.tile([128, TG], dt.float32, tag="y2sb")
                        nc.scalar.activation(y2sb[:], psy[:], COPY)
                        nc.sync.dma_start(
                            y2t[hc * 128:(hc + 1) * 128, g * TG:(g + 1) * TG], y2sb[:]
                        )
                    # gather: x2_save^T[j, t] = sum_i sel[i, j] * x2T[i, t]
                    pss = p2pss.tile([128, TG], dt.float32, tag="gatps")
                    for ic in range(ICH):
                        nc.tensor.matmul(
                            pss[:K, :], lhsT=sel[:, ic, :], rhs=x2h_sb[:, ic, :],
                            start=(ic == 0), stop=False,
                        )
                        nc.tensor.matmul(
                            pss[:K, :], lhsT=sel[:, ic, :], rhs=x2l_sb[:, ic, :],
                            start=False, stop=(ic == ICH - 1),
                        )
                    savesb = p2.tile([128, TG], dt.float32, tag="savesb")
                    nc.scalar.activation(savesb[:K, :], pss[:K, :], COPY)
                    nc.sync.dma_start(savet[:, g * TG:(g + 1) * TG], savesb[:K, :])

    nc.finalize()
    return nc


def _host_pack(x1, w_up, w_up_lora_a, w_up_lora_b, w_down, w_down_lora_a, w_down_lora_b):
    W_up = (w_up + w_up_lora_a @ w_up_lora_b).astype(np.float32)
    W_dn = (w_down + w_down_lora_a @ w_down_lora_b).astype(np.float32)

    Wup_hi, Wup_lo = _split(W_up)
    Wdn_hi, Wdn_lo = _split(W_dn)
    # [H, I] -> [ICH, 128h, HCH*128i]
    wup_hi = np.ascontiguousarray(
        Wup_hi.reshape(HCH, 128, ICH, 128).transpose(2, 1, 0, 3).reshape(ICH, 128, HCH * 128)
    )
    wup_lo = np.ascontiguousarray(
        Wup_lo.reshape(HCH, 128, ICH, 128).transpose(2, 1, 0, 3).reshape(ICH, 128, HCH * 128)
    )
    # [I, H] -> [HCH, 128i, ICH*128h]
    wdn_hi = np.ascontiguousarray(
        Wdn_hi.reshape(ICH, 128, HCH, 128).transpose(2, 1, 0, 3).reshape(HCH, 128, ICH * 128)
    )
    wdn_lo = np.ascontiguousarray(
        Wdn_lo.reshape(ICH, 128, HCH, 128).transpose(2, 1, 0, 3).reshape(HCH, 128, ICH * 128)
    )

    xf = np.ascontiguousarray(x1.reshape(B * S, HIDDEN))
    iota = (np.arange(128)[:, None] + 128 * np.arange(ICH)[None, :])
    iota_i = np.ascontiguousarray(iota.astype(np.int32))
    iota_f = np.ascontiguousarray(iota.astype(np.float32))

    in_maps = []
    for c in range(NCORES):
        xs = xf[c * TOK:(c + 1) * TOK]             # [TOK, H]
        xsT = np.ascontiguousarray(xs.T)           # [H, TOK]
        xh, xl = _split(xsT)
        in_maps.append(
            {
                "x1t_hi": np.ascontiguousarray(xh.reshape(HCH, 128, TOK)),
                "x1t_lo": np.ascontiguousarray(xl.reshape(HCH, 128, TOK)),
                "wup_hi": wup_hi,
                "wup_lo": wup_lo,
                "wdn_hi": wdn_hi,
                "wdn_lo": wdn_lo,
                "iota_i": iota_i,
                "iota_f": iota_f,
            }
        )
    return in_maps


def kernel(x1, w_up, w_up_lora_a, w_up_lora_b, w_down, w_down_lora_a, w_down_lora_b):
    from concourse.bass_utils import run_bass_kernel_spmd

    if "nc" not in _PROG_CACHE:
        _PROG_CACHE["nc"] = _build_program()
    nc = _PROG_CACHE["nc"]

    in_maps = _host_pack(
        np.asarray(x1, dtype=np.float32),
        np.asarray(w_up, dtype=np.float32),
        np.asarray(w_up_lora_a, dtype=np.float32),
        np.asarray(w_up_lora_b, dtype=np.float32),
        np.asarray(w_down, dtype=np.float32),
        np.asarray(w_down_lora_a, dtype=np.float32),
        np.asarray(w_down_lora_b, dtype=np.float32),
    )
    _PROG_CACHE["in_maps"] = in_maps

    res = run_bass_kernel_spmd(nc, in_maps, list(range(NCORES)))
    _PROG_CACHE["last_result"] = res

    y2 = np.empty((B * S, HIDDEN), dtype=np.float32)
    save = np.empty((B * S, K), dtype=np.float32)
    for c in range(NCORES):
        r = res.results[c]
        y2[c * TOK:(c + 1) * TOK] = r["y2t"].T
        save[c * TOK:(c + 1) * TOK] = r["savet"].T
    return y2.reshape(B, S, HIDDEN), save.reshape(B, S, K)
